# revision 9
# baseline (speedup 1.0000x reference)
"""Multi-head attention (B=4, S=2048, H=1024, 16 heads) on 8 TRN2 NeuronCores.

Sharding: core c handles (batch b = c//2, head-group g = c%2 of 8 heads).
Per-core device program (all activations kept feature-major, i.e. transposed):
  X^T  [1024,2048]  host-transposed bf16 query shard
  Q^T,K^T = Wq/k^T X^T            (PE, bf16, fp32 PSUM)
  V       = X W_v  (natural [s,d] via lhsT=X^T tiles)
  S^T  = per head, per k-tile: lhsT=K^T slice, rhs=Q^T slice (row-packed pairs)
  E    = exp(S^T/8)  on ScalarE, bf16 out
  O^T,sums = V'.T @ E  where V' = [V | 1] (ones column yields softmax sums)
  O^T /= sums (fast-approx reciprocal + gpsimd partition-broadcast + DVE mul)
  Y^T  = W_o^T O^T   -> DMA out per [128,512] tile

Schedule: qc (query-chunk) outer, head-pair inner.  V tiles and the next
pair's Q^T/K^T matmuls are interleaved into the attention st-loop so the
PE fills the slack under the ScalarE exp stream; the output projection for
chunk qc-1 runs under the attention of chunk qc, so only the last chunk's
projection trails.

Host: gathers per-core Y^T tiles, sums the two head-group partials per batch,
adds b_o. b_qkv / attention bias are zero in this problem; nonzero values are
still handled (extra rank-1 bias matmuls / DVE bias adds) via build flags.
"""

import numpy as np
import ml_dtypes

import concourse.bass as bass
import concourse.tile as tile
from concourse import bacc
import concourse.mybir as mybir
from concourse.bass_utils import run_bass_kernel_spmd

F32 = mybir.dt.float32
BF16 = mybir.dt.bfloat16
AF = mybir.ActivationFunctionType

HIDDEN = 1024
HEADS = 16
HD = 64
B = 4
S_FULL = 2048
NCORES = 8
HPG = HEADS // 2          # heads per group/core = 8
GF = HPG * HD             # group feature width = 512
NPAIR = HPG // 2          # head pairs per core = 4


def build_program(S=S_FULL, has_bqkv=False, has_bias=False):
    KT = HIDDEN // 128            # hidden k-tiles = 8
    CH = min(512, S)              # free-dim chunk
    NQC = S // CH                 # q chunks
    SKT = S // 128                # seq k-tiles (attention contraction)
    NM = HIDDEN // 128            # output-projection m-tiles = 8

    nc = bacc.Bacc(
        "TRN2",
        target_bir_lowering=False,
        debug=False,
        enable_asserts=False,
        num_devices=NCORES,
    )

    x_dram = nc.dram_tensor("x", [HIDDEN, S], BF16, kind="ExternalInput")  # X^T, host-transposed
    wqkv_dram = nc.dram_tensor("wqkv", [HIDDEN, 3 * GF], BF16, kind="ExternalInput")
    wo_dram = nc.dram_tensor("wo", [GF, HIDDEN], BF16, kind="ExternalInput")
    QK = 2 * GF               # q|k columns of the fused qkv weight
    if has_bqkv:
        bqkv_dram = nc.dram_tensor("bqkv", [1, 3 * GF], BF16, kind="ExternalInput")
    if has_bias:
        # host passes bias[0,0].T * 8 so exp(0.125*(S + bias8)) = exp(S/8 + bias)
        bias8_dram = nc.dram_tensor("bias8t", [S, S], F32, kind="ExternalInput")
    y_dram = nc.dram_tensor("y", [NM, NQC, 128, CH], F32, kind="ExternalOutput")

    with tile.TileContext(nc) as tc:
        with (
            tc.tile_pool(name="res", bufs=1) as res,
            tc.tile_pool(name="wrk", bufs=2) as wrk,
            tc.tile_pool(name="ep", bufs=6) as ep,
            tc.tile_pool(name="ps", bufs=2, space="PSUM") as ps,
        ):
            xt = res.tile([128, KT * S], BF16, tag="xt")
            wqkv = res.tile([128, KT * 3 * GF], BF16, tag="wqkv")
            wo = res.tile([128, (GF // 128) * HIDDEN], BF16, tag="wo")
            # V' tiles: per s-tile block of 8 heads x 65 cols (65th col = 1.0)
            vp = res.tile([128, SKT * HPG * 65], BF16, tag="vp")
            ot = res.tile([128, NPAIR * S], BF16, tag="ot")
            # Q^T/K^T for all four pairs stay live across the qc-outer loop
            qkt = res.tile([128, NPAIR * 2 * S], BF16, tag="qkt")

            nc.vector.memset(vp[:, :], 1.0)

            for kt in range(KT):
                nc.sync.dma_start(
                    xt[:, kt * S:(kt + 1) * S], x_dram[kt * 128:(kt + 1) * 128, :]
                )
                nc.scalar.dma_start(
                    wqkv[:, kt * 3 * GF:(kt + 1) * 3 * GF],
                    wqkv_dram[kt * 128:(kt + 1) * 128, :],
                )
            for ft in range(GF // 128):
                nc.scalar.dma_start(
                    wo[:, ft * HIDDEN:(ft + 1) * HIDDEN],
                    wo_dram[ft * 128:(ft + 1) * 128, :],
                )
            if has_bqkv:
                bq = res.tile([1, 3 * GF], BF16, tag="bq")
                nc.sync.dma_start(bq[:, :], bqkv_dram[:, :])
                ones = res.tile([1, CH], BF16, tag="ones")
                nc.vector.memset(ones[:, :], 1.0)

            def acc_matmul(out_ps, lhsT_of, rhs_of, bias_lhsT, bias_rhs):
                """Accumulate KT matmuls (+ optional rank-1 bias term) into PSUM."""
                if bias_lhsT is not None:
                    nc.tensor.matmul(out_ps, bias_lhsT, bias_rhs, start=True, stop=False)
                for kt in range(KT):
                    nc.tensor.matmul(
                        out_ps,
                        lhsT_of(kt),
                        rhs_of(kt),
                        start=(kt == 0 and bias_lhsT is None),
                        stop=(kt == KT - 1),
                    )

            def emit_v_tile(st):
                """V' (natural [s, d]) for s-tile st, all 8 heads."""
                vps = ps.tile([128, GF], F32, tag="acc")
                acc_matmul(
                    vps[:, :],
                    lambda kt, st=st: xt[:, kt * S + st * 128: kt * S + (st + 1) * 128],
                    lambda kt: wqkv[:, kt * 3 * GF + 2 * GF: kt * 3 * GF + 3 * GF],
                    ones[0:1, 0:128] if has_bqkv else None,
                    bq[0:1, 2 * GF:3 * GF] if has_bqkv else None,
                )
                dst = vp[:, st * HPG * 65:(st + 1) * HPG * 65]
                dst = dst.rearrange("p (h c) -> p h c", c=65)[:, :, 0:64]
                src = vps.rearrange("p (h c) -> p h c", c=64)
                nc.vector.tensor_copy(dst, src)

            def emit_qk_chunk(p, which, qc):
                """One CH-wide chunk of Q^T (which=0) or K^T (which=1) for pair p."""
                colbase = p * 128 if which == 0 else GF + p * 128
                dst = qkt[:, (2 * p + which) * S + qc * CH:(2 * p + which) * S + (qc + 1) * CH]
                qkps = ps.tile([128, CH], F32, tag="acc")
                acc_matmul(
                    qkps[:, :],
                    lambda kt, cb=colbase: wqkv[:, kt * 3 * GF + cb: kt * 3 * GF + cb + 128],
                    lambda kt, qc=qc: xt[:, kt * S + qc * CH: kt * S + (qc + 1) * CH],
                    bq[0:1, colbase:colbase + 128] if has_bqkv else None,
                    ones[0:1, 0:CH] if has_bqkv else None,
                )
                nc.vector.tensor_copy(dst, qkps[:, :])

            def emit_oproj(qc, ms):
                """Output-projection tiles m in ms for query chunk qc."""
                for m in ms:
                    yps = ps.tile([128, CH], F32, tag="acc")
                    for ft in range(GF // 128):
                        nc.tensor.matmul(
                            yps[:, :],
                            wo[:, ft * HIDDEN + m * 128: ft * HIDDEN + (m + 1) * 128],
                            ot[:, ft * S + qc * CH: ft * S + (qc + 1) * CH],
                            start=(ft == 0),
                            stop=(ft == GF // 128 - 1),
                        )
                    ysb = wrk.tile([128, CH], F32, tag="ysb", bufs=3)
                    nc.vector.tensor_copy(ysb[:, :], yps[:, :])
                    nc.sync.dma_start(y_dram[m, qc], ysb[:, :])

            def attention_unit(p, qc, extra_pe):
                """st-loop for (pair p, chunk qc); extra_pe(st) interleaves
                independent PE work (V tiles / next pair's QK / O-proj)."""
                qt = qkt[:, (2 * p + 0) * S:(2 * p + 1) * S]
                ktt = qkt[:, (2 * p + 1) * S:(2 * p + 2) * S]
                avs = []
                for j in (0, 1):
                    avp = ps.tile([65, CH], F32, tag="av", bufs=2)
                    avs.append(avp)

                def emit_av(e_tile, st):
                    for j in (0, 1):
                        h = p * 2 + j
                        nc.tensor.matmul(
                            avs[j][:, :],
                            vp[:, st * HPG * 65 + h * 65: st * HPG * 65 + (h + 1) * 65],
                            e_tile[:, j * CH:(j + 1) * CH],
                            start=(st == 0),
                            stop=(st == SKT - 1),
                        )

                pending = None  # (e_tile, st) — AV emitted one k-tile late
                for st in range(SKT):
                    e = ep.tile([128, 2 * CH], BF16, tag="e")
                    sp = ps.tile([128, 2 * CH], F32, tag="sp", bufs=2)
                    for j in (0, 1):
                        hs = slice(j * 64, (j + 1) * 64)
                        spv = sp[:, j * CH:(j + 1) * CH]
                        nc.tensor.matmul(
                            spv,
                            ktt[hs, st * 128:(st + 1) * 128],
                            qt[hs, qc * CH:(qc + 1) * CH],
                            start=True,
                            stop=True,
                            tile_position=(j * 64, 0),
                        )
                        if has_bias:
                            b8 = ep.tile([128, CH], F32, tag="b8", bufs=2)
                            nc.sync.dma_start(
                                b8[:, :],
                                bias8_dram[st * 128:(st + 1) * 128, qc * CH:(qc + 1) * CH],
                            )
                            nc.vector.tensor_add(spv, spv, b8[:, :])
                    nc.scalar.activation(e[:, :], sp[:, :], AF.Exp, scale=0.125)
                    if pending is not None:
                        emit_av(*pending)
                    extra_pe(st)
                    pending = (e, st)
                emit_av(*pending)

                # fast PSUM->SBUF copies release both accumulator slots
                # first; the normalize chain (recip/bcast/mul) then runs
                # off the PE critical path. GpSimd runs ONLY
                # partition_broadcast (mixing gpsimd op types causes a
                # ~6us library reload per op).
                raws = []
                for j in (0, 1):
                    raw = wrk.tile([65, CH], F32, tag="raw", bufs=4)
                    nc.vector.tensor_copy(raw[:, :], avs[j][:, :])
                    raws.append(raw)
                for j in (0, 1):
                    # custom-DVE/gpsimd ops require partition-base-0 sources:
                    # shift the sums row down with a plain copy first
                    s0 = wrk.tile([1, CH], F32, tag="s0")
                    nc.vector.tensor_copy(s0[:, :], raws[j][64:65, :])
                    rec = wrk.tile([1, CH], F32, tag="rec")
                    nc.vector.reciprocal_approx_fast(rec[:, :], s0[:, :])
                    bc = wrk.tile([64, CH], F32, tag="bc", bufs=4)
                    nc.gpsimd.partition_broadcast(bc[:, :], rec[:, :])
                    nc.vector.tensor_mul(
                        ot[j * 64:(j + 1) * 64, p * S + qc * CH: p * S + (qc + 1) * CH],
                        raws[j][0:64, :],
                        bc[:, :],
                    )

            # ---- qc-outer / pair-inner schedule ----
            # prologue: first pair's Q^T/K^T (chunk 0 of Q first so the first
            # logits matmul can issue as early as possible), all of K^T
            for qc0 in range(NQC):
                emit_qk_chunk(0, 1, qc0)     # K^T pair 0
            emit_qk_chunk(0, 0, 0)           # Q^T pair 0 chunk 0

            for qc in range(NQC):
                for p in range(NPAIR):
                    # interleaved filler work for the PE under the exp stream
                    filler = []
                    if qc == 0:
                        if p == 0:
                            # V' tile st must be ready before emit_av(st);
                            # AV lags one st so emit V[st] at st.
                            filler = [("v", st) for st in range(SKT)]
                            # remaining Q^T chunks of pair 0
                            filler += [("qk", 0, 0, c) for c in range(1, NQC)]
                        else:
                            pass
                        if p + 1 < NPAIR:
                            # next pair's full Q^T/K^T: 8 chunks
                            filler += [("qk", p + 1, w, c) for w in (1, 0) for c in range(NQC)]
                    else:
                        # O-projection of previous chunk: 8 m-tiles over 4 pairs
                        filler += [("o", qc - 1, m) for m in range(2 * p, 2 * p + 2)]

                    fi = iter(filler)
                    per_st = max(1, (len(filler) + SKT - 1) // SKT)

                    def extra_pe(st, fi=fi, per_st=per_st):
                        for _ in range(per_st):
                            w = next(fi, None)
                            if w is None:
                                return
                            if w[0] == "v":
                                emit_v_tile(w[1])
                            elif w[0] == "qk":
                                emit_qk_chunk(w[1], w[2], w[3])
                            else:
                                emit_oproj(w[1], [w[2]])

                    attention_unit(p, qc, extra_pe)
                    # drain any leftover filler
                    for w in fi:
                        if w[0] == "v":
                            emit_v_tile(w[1])
                        elif w[0] == "qk":
                            emit_qk_chunk(w[1], w[2], w[3])
                        else:
                            emit_oproj(w[1], [w[2]])
            emit_oproj(NQC - 1, range(NM))

    nc.compile()
    return nc


_BUILD_CACHE = {}


def _get_program(S, has_bqkv, has_bias):
    key = (S, has_bqkv, has_bias)
    if key not in _BUILD_CACHE:
        _BUILD_CACHE[key] = build_program(S, has_bqkv, has_bias)
    return _BUILD_CACHE[key]


def make_in_maps(query, bias, w_qkv, b_qkv, w_o, has_bqkv, has_bias):
    bf = ml_dtypes.bfloat16
    in_maps = []
    for c in range(NCORES):
        b, g = divmod(c, 2)
        cols = slice(g * GF, (g + 1) * GF)
        w_g = np.concatenate(
            [w_qkv[:, cols], w_qkv[:, HIDDEN:][:, cols], w_qkv[:, 2 * HIDDEN:][:, cols]],
            axis=1,
        )
        m = {
            "x": np.ascontiguousarray(query[b].T).astype(bf),
            "wqkv": np.ascontiguousarray(w_g).astype(bf),
            "wo": np.ascontiguousarray(w_o[cols]).astype(bf),
        }
        if has_bqkv:
            b_g = np.concatenate(
                [b_qkv[cols], b_qkv[HIDDEN:][cols], b_qkv[2 * HIDDEN:][cols]]
            )
            m["bqkv"] = b_g.reshape(1, 3 * GF).astype(bf)
        if has_bias:
            m["bias8t"] = np.ascontiguousarray(bias[0, 0].T * 8.0).astype(np.float32)
        in_maps.append(m)
    return in_maps


def assemble_output(results, b_o, S=S_FULL):
    NQC = S // min(512, S)
    out = np.zeros((B, S, HIDDEN), np.float32)
    for c in range(NCORES):
        b, _g = divmod(c, 2)
        y = results[c]["y"]  # [NM, NQC, 128, CH]
        yt = y.transpose(0, 2, 1, 3).reshape(HIDDEN, S)
        out[b] += yt.T
    out += np.asarray(b_o, np.float32)[None, None, :]
    return out


def kernel(query, bias, w_qkv, b_qkv, w_o, b_o, _trace=False):
    query = np.asarray(query, np.float32)
    bias = np.asarray(bias, np.float32)
    w_qkv = np.asarray(w_qkv, np.float32)
    b_qkv = np.asarray(b_qkv, np.float32)
    w_o = np.asarray(w_o, np.float32)
    b_o = np.asarray(b_o, np.float32)

    has_bqkv = bool(np.any(b_qkv))
    has_bias = bool(np.any(bias))
    nc = _get_program(S_FULL, has_bqkv, has_bias)
    in_maps = make_in_maps(query, bias, w_qkv, b_qkv, w_o, has_bqkv, has_bias)
    res = run_bass_kernel_spmd(
        nc, in_maps, core_ids=list(range(NCORES)), trace=_trace
    )
    out = assemble_output(res.results, b_o)
    if _trace:
        return out, res
    return out


# revision 15
# speedup vs baseline: 1.0564x; 1.0564x over previous
"""Multi-head attention (B=4, S=2048, H=1024, 16 heads) on 8 TRN2 NeuronCores.

Sharding: core c handles (batch b = c//2, head-group g = c%2 of 8 heads).
Per-core device program (all activations kept feature-major, i.e. transposed):
  X^T  [1024,2048]  host-transposed bf16 query shard
  Q^T,K^T = Wq/k^T X^T            (PE, bf16, fp32 PSUM)
  V       = X W_v  (natural [s,d] via lhsT=X^T tiles)
  S^T  = per head, per k-tile: lhsT=K^T slice, rhs=Q^T slice (row-packed pairs)
  E    = exp(S^T/8)  on ScalarE, bf16 out
  O^T,sums = V'.T @ E  where V' = [V | 1] (ones column yields softmax sums)
  O^T /= sums (fast-approx reciprocal + gpsimd partition-broadcast + DVE mul)
  Y^T  = W_o^T O^T   -> DMA out per [128,512] tile

Schedule: qc (query-chunk) outer, head-pair inner.  V tiles and the next
pair's Q^T/K^T matmuls are interleaved into the attention st-loop so the
PE fills the slack under the ScalarE exp stream; the output projection for
chunk qc-1 runs under the attention of chunk qc, so only the last chunk's
projection trails.

Host: gathers per-core Y^T tiles, sums the two head-group partials per batch,
adds b_o. b_qkv / attention bias are zero in this problem; nonzero values are
still handled (extra rank-1 bias matmuls / DVE bias adds) via build flags.
"""

import numpy as np
import ml_dtypes

import concourse.bass as bass
import concourse.tile as tile
from concourse import bacc
import concourse.mybir as mybir
from concourse.bass_utils import run_bass_kernel_spmd

F32 = mybir.dt.float32
BF16 = mybir.dt.bfloat16
AF = mybir.ActivationFunctionType

HIDDEN = 1024
HEADS = 16
HD = 64
B = 4
S_FULL = 2048
NCORES = 8
HPG = HEADS // 2          # heads per group/core = 8
GF = HPG * HD             # group feature width = 512
NPAIR = HPG // 2          # head pairs per core = 4


def build_program(S=S_FULL, has_bqkv=False, has_bias=False):
    KT = HIDDEN // 128            # hidden k-tiles = 8
    CH = min(512, S)              # free-dim chunk
    NQC = S // CH                 # q chunks
    SKT = S // 128                # seq k-tiles (attention contraction)
    NM = HIDDEN // 128            # output-projection m-tiles = 8

    nc = bacc.Bacc(
        "TRN2",
        target_bir_lowering=False,
        debug=False,
        enable_asserts=False,
        num_devices=NCORES,
    )

    x_dram = nc.dram_tensor("x", [HIDDEN, S], BF16, kind="ExternalInput")  # X^T, host-transposed
    wqkv_dram = nc.dram_tensor("wqkv", [HIDDEN, 3 * GF], BF16, kind="ExternalInput")
    wo_dram = nc.dram_tensor("wo", [GF, HIDDEN], BF16, kind="ExternalInput")
    QK = 2 * GF               # q|k columns of the fused qkv weight
    if has_bqkv:
        bqkv_dram = nc.dram_tensor("bqkv", [1, 3 * GF], BF16, kind="ExternalInput")
    if has_bias:
        # host passes bias[0,0].T * 8 so exp(0.125*(S + bias8)) = exp(S/8 + bias)
        bias8_dram = nc.dram_tensor("bias8t", [S, S], F32, kind="ExternalInput")
    y_dram = nc.dram_tensor("y", [NM, NQC, 128, CH], F32, kind="ExternalOutput")

    with tile.TileContext(nc) as tc:
        with (
            tc.tile_pool(name="res", bufs=1) as res,
            tc.tile_pool(name="wrk", bufs=2) as wrk,
            tc.tile_pool(name="ep", bufs=6) as ep,
            tc.tile_pool(name="ps", bufs=2, space="PSUM") as ps,
        ):
            xt = res.tile([128, KT * S], BF16, tag="xt")
            wqkv = res.tile([128, KT * 3 * GF], BF16, tag="wqkv")
            wo = res.tile([128, (GF // 128) * HIDDEN], BF16, tag="wo")
            # V' tiles: per s-tile block of 8 heads x 65 cols (65th col = 1.0)
            vp = res.tile([128, SKT * HPG * 65], BF16, tag="vp")
            ot = res.tile([128, NPAIR * S], BF16, tag="ot")
            # Q^T/K^T for all four pairs stay live across the qc-outer loop
            qkt = res.tile([128, NPAIR * 2 * S], BF16, tag="qkt")

            nc.vector.memset(vp[:, :], 1.0)

            # q|k weight columns land first so the first logits tile can
            # issue before the v columns / wo finish streaming in
            for kt in range(KT):
                nc.sync.dma_start(
                    xt[:, kt * S:(kt + 1) * S], x_dram[kt * 128:(kt + 1) * 128, :]
                )
                nc.scalar.dma_start(
                    wqkv[:, kt * 3 * GF:kt * 3 * GF + QK],
                    wqkv_dram[kt * 128:(kt + 1) * 128, 0:QK],
                )
            for kt in range(KT):
                nc.scalar.dma_start(
                    wqkv[:, kt * 3 * GF + QK:(kt + 1) * 3 * GF],
                    wqkv_dram[kt * 128:(kt + 1) * 128, QK:3 * GF],
                )
            for ft in range(GF // 128):
                nc.scalar.dma_start(
                    wo[:, ft * HIDDEN:(ft + 1) * HIDDEN],
                    wo_dram[ft * 128:(ft + 1) * 128, :],
                )
            if has_bqkv:
                bq = res.tile([1, 3 * GF], BF16, tag="bq")
                nc.sync.dma_start(bq[:, :], bqkv_dram[:, :])
                ones = res.tile([1, CH], BF16, tag="ones")
                nc.vector.memset(ones[:, :], 1.0)

            def acc_matmul(out_ps, lhsT_of, rhs_of, bias_lhsT, bias_rhs):
                """Accumulate KT matmuls (+ optional rank-1 bias term) into PSUM."""
                if bias_lhsT is not None:
                    nc.tensor.matmul(out_ps, bias_lhsT, bias_rhs, start=True, stop=False)
                for kt in range(KT):
                    nc.tensor.matmul(
                        out_ps,
                        lhsT_of(kt),
                        rhs_of(kt),
                        start=(kt == 0 and bias_lhsT is None),
                        stop=(kt == KT - 1),
                    )

            def emit_v_tile(st):
                """V' (natural [s, d]) for s-tile st, all 8 heads."""
                vps = ps.tile([128, GF], F32, tag="acc")
                acc_matmul(
                    vps[:, :],
                    lambda kt, st=st: xt[:, kt * S + st * 128: kt * S + (st + 1) * 128],
                    lambda kt: wqkv[:, kt * 3 * GF + 2 * GF: kt * 3 * GF + 3 * GF],
                    ones[0:1, 0:128] if has_bqkv else None,
                    bq[0:1, 2 * GF:3 * GF] if has_bqkv else None,
                )
                dst = vp[:, st * HPG * 65:(st + 1) * HPG * 65]
                dst = dst.rearrange("p (h c) -> p h c", c=65)[:, :, 0:64]
                src = vps.rearrange("p (h c) -> p h c", c=64)
                nc.vector.tensor_copy(dst, src)

            def emit_qk_chunk(p, which, qc):
                """One CH-wide chunk of Q^T (which=0) or K^T (which=1) for pair p."""
                colbase = p * 128 if which == 0 else GF + p * 128
                dst = qkt[:, (2 * p + which) * S + qc * CH:(2 * p + which) * S + (qc + 1) * CH]
                qkps = ps.tile([128, CH], F32, tag="acc")
                acc_matmul(
                    qkps[:, :],
                    lambda kt, cb=colbase: wqkv[:, kt * 3 * GF + cb: kt * 3 * GF + cb + 128],
                    lambda kt, qc=qc: xt[:, kt * S + qc * CH: kt * S + (qc + 1) * CH],
                    bq[0:1, colbase:colbase + 128] if has_bqkv else None,
                    ones[0:1, 0:CH] if has_bqkv else None,
                )
                nc.vector.tensor_copy(dst, qkps[:, :])

            def emit_oproj(qc, ms):
                """Output-projection tiles m in ms for query chunk qc."""
                for m in ms:
                    yps = ps.tile([128, CH], F32, tag="acc")
                    for ft in range(GF // 128):
                        nc.tensor.matmul(
                            yps[:, :],
                            wo[:, ft * HIDDEN + m * 128: ft * HIDDEN + (m + 1) * 128],
                            ot[:, ft * S + qc * CH: ft * S + (qc + 1) * CH],
                            start=(ft == 0),
                            stop=(ft == GF // 128 - 1),
                        )
                    ysb = wrk.tile([128, CH], F32, tag="ysb", bufs=3)
                    nc.vector.tensor_copy(ysb[:, :], yps[:, :])
                    nc.sync.dma_start(y_dram[m, qc], ysb[:, :])

            AV_LAG = 4  # AV trails the exp stream; must stay < ep pool bufs - 1

            def attention_unit(p, qc, extra_pe):
                """st-loop for (pair p, chunk qc); extra_pe(st) interleaves
                independent PE work (V tiles / next pair's QK / O-proj)."""
                qt = qkt[:, (2 * p + 0) * S:(2 * p + 1) * S]
                ktt = qkt[:, (2 * p + 1) * S:(2 * p + 2) * S]
                avs = []
                for j in (0, 1):
                    avp = ps.tile([65, CH], F32, tag="av", bufs=2)
                    avs.append(avp)

                def emit_av(e_tile, st):
                    # NOTE: splitting this into two row-packed K=64 halves
                    # accumulating the same PSUM (tile_position + start/stop
                    # group) hangs the device on this stack — keep K=128.
                    for j in (0, 1):
                        h = p * 2 + j
                        nc.tensor.matmul(
                            avs[j][:, :],
                            vp[:, st * HPG * 65 + h * 65: st * HPG * 65 + (h + 1) * 65],
                            e_tile[:, j * CH:(j + 1) * CH],
                            start=(st == 0),
                            stop=(st == SKT - 1),
                        )

                pending = []  # (e_tile, st) — AV trails by AV_LAG k-tiles
                for st in range(SKT):
                    if len(pending) >= AV_LAG:
                        emit_av(*pending.pop(0))
                    if st > 0:
                        # ahead of the logits emission so the in-order PE
                        # queue can chew filler while logits waits on ACT
                        extra_pe(st)
                    e = ep.tile([128, 2 * CH], BF16, tag="e")
                    sp = ps.tile([128, 2 * CH], F32, tag="sp", bufs=2)
                    for j in (0, 1):
                        hs = slice(j * 64, (j + 1) * 64)
                        spv = sp[:, j * CH:(j + 1) * CH]
                        nc.tensor.matmul(
                            spv,
                            ktt[hs, st * 128:(st + 1) * 128],
                            qt[hs, qc * CH:(qc + 1) * CH],
                            start=True,
                            stop=True,
                            tile_position=(j * 64, 0),
                        )
                        if has_bias:
                            b8 = ep.tile([128, CH], F32, tag="b8", bufs=2)
                            nc.sync.dma_start(
                                b8[:, :],
                                bias8_dram[st * 128:(st + 1) * 128, qc * CH:(qc + 1) * CH],
                            )
                            nc.vector.tensor_add(spv, spv, b8[:, :])
                    nc.scalar.activation(e[:, :], sp[:, :], AF.Exp, scale=0.125)
                    if st == 0:
                        extra_pe(st)
                    pending.append((e, st))
                for ev in pending:
                    emit_av(*ev)

                # fast PSUM->SBUF copies release both accumulator slots
                # first; the normalize chain (recip/bcast/mul) then runs
                # off the PE critical path. GpSimd runs ONLY
                # partition_broadcast (mixing gpsimd op types causes a
                # ~6us library reload per op).
                raws = []
                for j in (0, 1):
                    raw = wrk.tile([65, CH], F32, tag="raw", bufs=4)
                    nc.vector.tensor_copy(raw[:, :], avs[j][:, :])
                    raws.append(raw)
                for j in (0, 1):
                    # custom-DVE/gpsimd ops require partition-base-0 sources:
                    # shift the sums row down with a plain copy first
                    s0 = wrk.tile([1, CH], F32, tag="s0")
                    nc.vector.tensor_copy(s0[:, :], raws[j][64:65, :])
                    rec = wrk.tile([1, CH], F32, tag="rec")
                    nc.vector.reciprocal_approx_fast(rec[:, :], s0[:, :])
                    bc = wrk.tile([64, CH], F32, tag="bc", bufs=4)
                    nc.gpsimd.partition_broadcast(bc[:, :], rec[:, :])
                    nc.vector.tensor_mul(
                        ot[j * 64:(j + 1) * 64, p * S + qc * CH: p * S + (qc + 1) * CH],
                        raws[j][0:64, :],
                        bc[:, :],
                    )

            # ---- qc-outer / pair-inner schedule ----
            # prologue: only what the very first logits tile needs — K^T and
            # Q^T chunk 0 of pair 0; everything else rides in unit fillers
            emit_qk_chunk(0, 1, 0)
            emit_qk_chunk(0, 0, 0)

            def run_filler(w):
                if w[0] == "v":
                    emit_v_tile(w[1])
                elif w[0] == "qk":
                    emit_qk_chunk(w[1], w[2], w[3])
                else:
                    emit_oproj(w[1], [w[2]])

            for qc in range(NQC):
                for p in range(NPAIR):
                    filler = []
                    if qc == 0:
                        if p == 0:
                            # rest of K^T(p0): chunk c covers logits st 4c..4c+3
                            filler += [("qk", 0, 1, c) for c in range(1, NQC)]
                            # V' tiles: AV trails by AV_LAG so V[st] emitted
                            # around st/2 is always ready in time
                            filler += [("v", st) for st in range(SKT)]
                        if p + 1 < NPAIR:
                            # next pair's K^T and first Q^T chunk
                            filler += [("qk", p + 1, 1, c) for c in range(NQC)]
                            filler += [("qk", p + 1, 0, 0)]
                        # this pair's next Q^T chunk (needed at qc=1)
                        filler += [("qk", p, 0, 1)]
                    else:
                        # O-projection of previous chunk: 2 m-tiles per unit
                        filler += [("o", qc - 1, m) for m in range(2 * p, 2 * p + 2)]
                        if qc + 1 < NQC:
                            filler += [("qk", p, 0, qc + 1)]

                    fi = iter(filler)
                    per_st = max(1, (len(filler) + SKT - 3) // (SKT - 2))

                    def extra_pe(st, fi=fi, per_st=per_st):
                        for _ in range(per_st):
                            w = next(fi, None)
                            if w is None:
                                return
                            run_filler(w)

                    attention_unit(p, qc, extra_pe)
                    for w in fi:
                        run_filler(w)
            emit_oproj(NQC - 1, range(NM))

    nc.compile()
    return nc


_BUILD_CACHE = {}


def _get_program(S, has_bqkv, has_bias):
    key = (S, has_bqkv, has_bias)
    if key not in _BUILD_CACHE:
        _BUILD_CACHE[key] = build_program(S, has_bqkv, has_bias)
    return _BUILD_CACHE[key]


def make_in_maps(query, bias, w_qkv, b_qkv, w_o, has_bqkv, has_bias):
    bf = ml_dtypes.bfloat16
    in_maps = []
    for c in range(NCORES):
        b, g = divmod(c, 2)
        cols = slice(g * GF, (g + 1) * GF)
        w_g = np.concatenate(
            [w_qkv[:, cols], w_qkv[:, HIDDEN:][:, cols], w_qkv[:, 2 * HIDDEN:][:, cols]],
            axis=1,
        )
        m = {
            "x": np.ascontiguousarray(query[b].T).astype(bf),
            "wqkv": np.ascontiguousarray(w_g).astype(bf),
            "wo": np.ascontiguousarray(w_o[cols]).astype(bf),
        }
        if has_bqkv:
            b_g = np.concatenate(
                [b_qkv[cols], b_qkv[HIDDEN:][cols], b_qkv[2 * HIDDEN:][cols]]
            )
            m["bqkv"] = b_g.reshape(1, 3 * GF).astype(bf)
        if has_bias:
            m["bias8t"] = np.ascontiguousarray(bias[0, 0].T * 8.0).astype(np.float32)
        in_maps.append(m)
    return in_maps


def assemble_output(results, b_o, S=S_FULL):
    NQC = S // min(512, S)
    out = np.zeros((B, S, HIDDEN), np.float32)
    for c in range(NCORES):
        b, _g = divmod(c, 2)
        y = results[c]["y"]  # [NM, NQC, 128, CH]
        yt = y.transpose(0, 2, 1, 3).reshape(HIDDEN, S)
        out[b] += yt.T
    out += np.asarray(b_o, np.float32)[None, None, :]
    return out


def kernel(query, bias, w_qkv, b_qkv, w_o, b_o, _trace=False):
    query = np.asarray(query, np.float32)
    bias = np.asarray(bias, np.float32)
    w_qkv = np.asarray(w_qkv, np.float32)
    b_qkv = np.asarray(b_qkv, np.float32)
    w_o = np.asarray(w_o, np.float32)
    b_o = np.asarray(b_o, np.float32)

    has_bqkv = bool(np.any(b_qkv))
    has_bias = bool(np.any(bias))
    nc = _get_program(S_FULL, has_bqkv, has_bias)
    in_maps = make_in_maps(query, bias, w_qkv, b_qkv, w_o, has_bqkv, has_bias)
    res = run_bass_kernel_spmd(
        nc, in_maps, core_ids=list(range(NCORES)), trace=_trace
    )
    out = assemble_output(res.results, b_o)
    if _trace:
        return out, res
    return out


# revision 18
# speedup vs baseline: 1.0894x; 1.0313x over previous
"""Multi-head attention (B=4, S=2048, H=1024, 16 heads) on 8 TRN2 NeuronCores.

Sharding: core c handles (batch b = c//2, head-group g = c%2 of 8 heads).
Per-core device program (all activations kept feature-major, i.e. transposed):
  X^T  [1024,2048]  host-transposed bf16 query shard
  Q^T,K^T = Wq/k^T X^T            (PE, bf16, fp32 PSUM)
  V       = X W_v  (natural [s,d] via lhsT=X^T tiles)
  S^T  = per head, per k-tile: lhsT=K^T slice, rhs=Q^T slice (row-packed pairs)
  E    = exp(S^T/8)  on ScalarE, bf16 out
  O^T,sums = V'.T @ E  where V' = [V | 1] (ones column yields softmax sums)
  O^T /= sums (fast-approx reciprocal + gpsimd partition-broadcast + DVE mul)
  Y^T  = W_o^T O^T   -> DMA out per [128,512] tile

Schedule: qc (query-chunk) outer, head-pair inner.  V tiles and the next
pair's Q^T/K^T matmuls are interleaved into the attention st-loop so the
PE fills the slack under the ScalarE exp stream; the output projection for
chunk qc-1 runs under the attention of chunk qc, so only the last chunk's
projection trails.

Host: gathers per-core Y^T tiles, sums the two head-group partials per batch,
adds b_o. b_qkv / attention bias are zero in this problem; nonzero values are
still handled (extra rank-1 bias matmuls / DVE bias adds) via build flags.
"""

import numpy as np
import ml_dtypes

import concourse.bass as bass
import concourse.tile as tile
from concourse import bacc
import concourse.mybir as mybir
from concourse.bass_utils import run_bass_kernel_spmd

F32 = mybir.dt.float32
BF16 = mybir.dt.bfloat16
AF = mybir.ActivationFunctionType

HIDDEN = 1024
HEADS = 16
HD = 64
B = 4
S_FULL = 2048
NCORES = 8
HPG = HEADS // 2          # heads per group/core = 8
GF = HPG * HD             # group feature width = 512
NPAIR = HPG // 2          # head pairs per core = 4


def build_program(S=S_FULL, has_bqkv=False, has_bias=False):
    KT = HIDDEN // 128            # hidden k-tiles = 8
    CH = min(512, S)              # free-dim chunk
    NQC = S // CH                 # q chunks
    SKT = S // 128                # seq k-tiles (attention contraction)
    NM = HIDDEN // 128            # output-projection m-tiles = 8

    nc = bacc.Bacc(
        "TRN2",
        target_bir_lowering=False,
        debug=False,
        enable_asserts=False,
        num_devices=NCORES,
    )

    x_dram = nc.dram_tensor("x", [HIDDEN, S], BF16, kind="ExternalInput")  # X^T, host-transposed
    wqkv_dram = nc.dram_tensor("wqkv", [HIDDEN, 3 * GF], BF16, kind="ExternalInput")
    wo_dram = nc.dram_tensor("wo", [GF, HIDDEN], BF16, kind="ExternalInput")
    QK = 2 * GF               # q|k columns of the fused qkv weight
    if has_bqkv:
        bqkv_dram = nc.dram_tensor("bqkv", [1, 3 * GF], BF16, kind="ExternalInput")
    if has_bias:
        # host passes bias[0,0].T * 8 so exp(0.125*(S + bias8)) = exp(S/8 + bias)
        bias8_dram = nc.dram_tensor("bias8t", [S, S], F32, kind="ExternalInput")
    y_dram = nc.dram_tensor("y", [NM, NQC, 128, CH], F32, kind="ExternalOutput")

    with tile.TileContext(nc) as tc:
        with (
            tc.tile_pool(name="res", bufs=1) as res,
            tc.tile_pool(name="wrk", bufs=2) as wrk,
            tc.tile_pool(name="ep", bufs=6) as ep,
            tc.tile_pool(name="ps", bufs=2, space="PSUM") as ps,
        ):
            xt = res.tile([128, KT * S], BF16, tag="xt")
            wqkv = res.tile([128, KT * 3 * GF], BF16, tag="wqkv")
            wo = res.tile([128, (GF // 128) * HIDDEN], BF16, tag="wo")
            # V' tiles: per s-tile block of 8 heads x 65 cols (65th col = 1.0)
            vp = res.tile([128, SKT * HPG * 65], BF16, tag="vp")
            ot = res.tile([128, NPAIR * S], BF16, tag="ot")
            # Q^T/K^T for all four pairs stay live across the qc-outer loop
            qkt = res.tile([128, NPAIR * 2 * S], BF16, tag="qkt")

            nc.vector.memset(vp[:, :], 1.0)

            # DMA order follows the dependency chain of the first logits
            # tile: x chunk 0 + pair-0 q/k weight columns first (1.5 MB),
            # everything else streams in behind.  Weights ride the gpsimd
            # queue; scalar (exp) and tensor queues stay clear.
            for kt in range(KT):
                rows = slice(kt * 128, (kt + 1) * 128)
                nc.sync.dma_start(xt[:, kt * S:kt * S + CH], x_dram[rows, 0:CH])
                nc.gpsimd.dma_start(
                    wqkv[:, kt * 3 * GF:kt * 3 * GF + 128], wqkv_dram[rows, 0:128]
                )
                nc.gpsimd.dma_start(
                    wqkv[:, kt * 3 * GF + GF:kt * 3 * GF + GF + 128],
                    wqkv_dram[rows, GF:GF + 128],
                )
            for kt in range(KT):
                rows = slice(kt * 128, (kt + 1) * 128)
                nc.sync.dma_start(xt[:, kt * S + CH:(kt + 1) * S], x_dram[rows, CH:S])
                nc.gpsimd.dma_start(
                    wqkv[:, kt * 3 * GF + 2 * GF:(kt + 1) * 3 * GF],
                    wqkv_dram[rows, 2 * GF:3 * GF],
                )
            for kt in range(KT):
                rows = slice(kt * 128, (kt + 1) * 128)
                nc.gpsimd.dma_start(
                    wqkv[:, kt * 3 * GF + GF + 128:kt * 3 * GF + 2 * GF],
                    wqkv_dram[rows, GF + 128:2 * GF],
                )
                nc.gpsimd.dma_start(
                    wqkv[:, kt * 3 * GF + 128:kt * 3 * GF + GF],
                    wqkv_dram[rows, 128:GF],
                )
            for ft in range(GF // 128):
                nc.gpsimd.dma_start(
                    wo[:, ft * HIDDEN:(ft + 1) * HIDDEN],
                    wo_dram[ft * 128:(ft + 1) * 128, :],
                )
            if has_bqkv:
                bq = res.tile([1, 3 * GF], BF16, tag="bq")
                nc.sync.dma_start(bq[:, :], bqkv_dram[:, :])
                ones = res.tile([1, CH], BF16, tag="ones")
                nc.vector.memset(ones[:, :], 1.0)

            def acc_matmul(out_ps, lhsT_of, rhs_of, bias_lhsT, bias_rhs):
                """Accumulate KT matmuls (+ optional rank-1 bias term) into PSUM."""
                if bias_lhsT is not None:
                    nc.tensor.matmul(out_ps, bias_lhsT, bias_rhs, start=True, stop=False)
                for kt in range(KT):
                    nc.tensor.matmul(
                        out_ps,
                        lhsT_of(kt),
                        rhs_of(kt),
                        start=(kt == 0 and bias_lhsT is None),
                        stop=(kt == KT - 1),
                    )

            def emit_v_tile(st):
                """V' (natural [s, d]) for s-tile st, all 8 heads."""
                vps = ps.tile([128, GF], F32, tag="acc")
                acc_matmul(
                    vps[:, :],
                    lambda kt, st=st: xt[:, kt * S + st * 128: kt * S + (st + 1) * 128],
                    lambda kt: wqkv[:, kt * 3 * GF + 2 * GF: kt * 3 * GF + 3 * GF],
                    ones[0:1, 0:128] if has_bqkv else None,
                    bq[0:1, 2 * GF:3 * GF] if has_bqkv else None,
                )
                dst = vp[:, st * HPG * 65:(st + 1) * HPG * 65]
                dst = dst.rearrange("p (h c) -> p h c", c=65)[:, :, 0:64]
                src = vps.rearrange("p (h c) -> p h c", c=64)
                nc.vector.tensor_copy(dst, src)

            def emit_qk_chunk(p, which, qc):
                """One CH-wide chunk of Q^T (which=0) or K^T (which=1) for pair p."""
                colbase = p * 128 if which == 0 else GF + p * 128
                dst = qkt[:, (2 * p + which) * S + qc * CH:(2 * p + which) * S + (qc + 1) * CH]
                qkps = ps.tile([128, CH], F32, tag="acc")
                acc_matmul(
                    qkps[:, :],
                    lambda kt, cb=colbase: wqkv[:, kt * 3 * GF + cb: kt * 3 * GF + cb + 128],
                    lambda kt, qc=qc: xt[:, kt * S + qc * CH: kt * S + (qc + 1) * CH],
                    bq[0:1, colbase:colbase + 128] if has_bqkv else None,
                    ones[0:1, 0:CH] if has_bqkv else None,
                )
                nc.vector.tensor_copy(dst, qkps[:, :])

            def emit_oproj(qc, ms):
                """Output-projection tiles m in ms for query chunk qc."""
                for m in ms:
                    yps = ps.tile([128, CH], F32, tag="acc")
                    for ft in range(GF // 128):
                        nc.tensor.matmul(
                            yps[:, :],
                            wo[:, ft * HIDDEN + m * 128: ft * HIDDEN + (m + 1) * 128],
                            ot[:, ft * S + qc * CH: ft * S + (qc + 1) * CH],
                            start=(ft == 0),
                            stop=(ft == GF // 128 - 1),
                        )
                    ysb = wrk.tile([128, CH], F32, tag="ysb", bufs=3)
                    nc.vector.tensor_copy(ysb[:, :], yps[:, :])
                    nc.sync.dma_start(y_dram[m, qc], ysb[:, :])

            AV_LAG = 4  # AV trails the exp stream; must stay < ep pool bufs - 1

            def attention_unit(p, qc, extra_pe):
                """st-loop for (pair p, chunk qc); extra_pe(st) interleaves
                independent PE work (V tiles / next pair's QK / O-proj)."""
                qt = qkt[:, (2 * p + 0) * S:(2 * p + 1) * S]
                ktt = qkt[:, (2 * p + 1) * S:(2 * p + 2) * S]
                avs = []
                for j in (0, 1):
                    avp = ps.tile([65, CH], F32, tag="av", bufs=2)
                    avs.append(avp)

                def emit_av(e_tile, st):
                    # NOTE: splitting this into two row-packed K=64 halves
                    # accumulating the same PSUM (tile_position + start/stop
                    # group) hangs the device on this stack — keep K=128.
                    for j in (0, 1):
                        h = p * 2 + j
                        nc.tensor.matmul(
                            avs[j][:, :],
                            vp[:, st * HPG * 65 + h * 65: st * HPG * 65 + (h + 1) * 65],
                            e_tile[:, j * CH:(j + 1) * CH],
                            start=(st == 0),
                            stop=(st == SKT - 1),
                        )

                pending = []  # (e_tile, st) — AV trails by AV_LAG k-tiles
                for st in range(SKT):
                    if len(pending) >= AV_LAG:
                        emit_av(*pending.pop(0))
                    if st > 0:
                        # ahead of the logits emission so the in-order PE
                        # queue can chew filler while logits waits on ACT
                        extra_pe(st)
                    e = ep.tile([128, 2 * CH], BF16, tag="e")
                    sp = ps.tile([128, 2 * CH], F32, tag="sp", bufs=2)
                    for j in (0, 1):
                        hs = slice(j * 64, (j + 1) * 64)
                        spv = sp[:, j * CH:(j + 1) * CH]
                        nc.tensor.matmul(
                            spv,
                            ktt[hs, st * 128:(st + 1) * 128],
                            qt[hs, qc * CH:(qc + 1) * CH],
                            start=True,
                            stop=True,
                            tile_position=(j * 64, 0),
                        )
                        if has_bias:
                            b8 = ep.tile([128, CH], F32, tag="b8", bufs=2)
                            nc.sync.dma_start(
                                b8[:, :],
                                bias8_dram[st * 128:(st + 1) * 128, qc * CH:(qc + 1) * CH],
                            )
                            nc.vector.tensor_add(spv, spv, b8[:, :])
                    nc.scalar.activation(e[:, :], sp[:, :], AF.Exp, scale=0.125)
                    if st == 0:
                        extra_pe(st)
                    pending.append((e, st))
                for ev in pending:
                    emit_av(*ev)

                # fast PSUM->SBUF copies release both accumulator slots
                # first; the normalize chain (recip/bcast/mul) then runs
                # off the PE critical path. GpSimd runs ONLY
                # partition_broadcast (mixing gpsimd op types causes a
                # ~6us library reload per op).
                raws = []
                for j in (0, 1):
                    raw = wrk.tile([65, CH], F32, tag="raw", bufs=4)
                    nc.vector.tensor_copy(raw[:, :], avs[j][:, :])
                    raws.append(raw)
                for j in (0, 1):
                    # custom-DVE/gpsimd ops require partition-base-0 sources:
                    # shift the sums row down with a plain copy first
                    s0 = wrk.tile([1, CH], F32, tag="s0")
                    nc.vector.tensor_copy(s0[:, :], raws[j][64:65, :])
                    rec = wrk.tile([1, CH], F32, tag="rec")
                    nc.vector.reciprocal_approx_fast(rec[:, :], s0[:, :])
                    bc = wrk.tile([64, CH], F32, tag="bc", bufs=4)
                    nc.gpsimd.partition_broadcast(bc[:, :], rec[:, :])
                    nc.vector.tensor_mul(
                        ot[j * 64:(j + 1) * 64, p * S + qc * CH: p * S + (qc + 1) * CH],
                        raws[j][0:64, :],
                        bc[:, :],
                    )

            # ---- qc-outer / pair-inner schedule ----
            # prologue: only what the very first logits tile needs — K^T and
            # Q^T chunk 0 of pair 0; everything else rides in unit fillers
            emit_qk_chunk(0, 1, 0)
            emit_qk_chunk(0, 0, 0)

            def run_filler(w):
                if w[0] == "v":
                    emit_v_tile(w[1])
                elif w[0] == "qk":
                    emit_qk_chunk(w[1], w[2], w[3])
                else:
                    emit_oproj(w[1], [w[2]])

            for qc in range(NQC):
                for p in range(NPAIR):
                    filler = []
                    if qc == 0:
                        if p == 0:
                            # rest of K^T(p0): chunk c covers logits st 4c..4c+3
                            filler += [("qk", 0, 1, c) for c in range(1, NQC)]
                            # V' tiles: AV trails by AV_LAG so V[st] emitted
                            # around st/2 is always ready in time
                            filler += [("v", st) for st in range(SKT)]
                        if p + 1 < NPAIR:
                            # next pair's K^T and first Q^T chunk
                            filler += [("qk", p + 1, 1, c) for c in range(NQC)]
                            filler += [("qk", p + 1, 0, 0)]
                        # this pair's next Q^T chunk (needed at qc=1)
                        filler += [("qk", p, 0, 1)]
                    else:
                        # O-projection of previous chunk: 2 m-tiles per unit
                        filler += [("o", qc - 1, m) for m in range(2 * p, 2 * p + 2)]
                        if qc + 1 < NQC:
                            filler += [("qk", p, 0, qc + 1)]

                    # O-proj fillers wait on the previous chunk's normalize
                    # chain — start them a few st in so they don't block the
                    # in-order PE queue ahead of this unit's first logits
                    f_start = 3 if qc > 0 else 0
                    fi = iter(filler)
                    per_st = max(1, (len(filler) + SKT - f_start - 3) // (SKT - f_start - 2))

                    def extra_pe(st, fi=fi, per_st=per_st, f_start=f_start):
                        if st < f_start:
                            return
                        for _ in range(per_st):
                            w = next(fi, None)
                            if w is None:
                                return
                            run_filler(w)

                    attention_unit(p, qc, extra_pe)
                    for w in fi:
                        run_filler(w)
            emit_oproj(NQC - 1, range(NM))

    nc.compile()
    return nc


_BUILD_CACHE = {}


def _get_program(S, has_bqkv, has_bias):
    key = (S, has_bqkv, has_bias)
    if key not in _BUILD_CACHE:
        _BUILD_CACHE[key] = build_program(S, has_bqkv, has_bias)
    return _BUILD_CACHE[key]


def make_in_maps(query, bias, w_qkv, b_qkv, w_o, has_bqkv, has_bias):
    bf = ml_dtypes.bfloat16
    in_maps = []
    for c in range(NCORES):
        b, g = divmod(c, 2)
        cols = slice(g * GF, (g + 1) * GF)
        w_g = np.concatenate(
            [w_qkv[:, cols], w_qkv[:, HIDDEN:][:, cols], w_qkv[:, 2 * HIDDEN:][:, cols]],
            axis=1,
        )
        m = {
            "x": np.ascontiguousarray(query[b].T).astype(bf),
            "wqkv": np.ascontiguousarray(w_g).astype(bf),
            "wo": np.ascontiguousarray(w_o[cols]).astype(bf),
        }
        if has_bqkv:
            b_g = np.concatenate(
                [b_qkv[cols], b_qkv[HIDDEN:][cols], b_qkv[2 * HIDDEN:][cols]]
            )
            m["bqkv"] = b_g.reshape(1, 3 * GF).astype(bf)
        if has_bias:
            m["bias8t"] = np.ascontiguousarray(bias[0, 0].T * 8.0).astype(np.float32)
        in_maps.append(m)
    return in_maps


def assemble_output(results, b_o, S=S_FULL):
    NQC = S // min(512, S)
    out = np.zeros((B, S, HIDDEN), np.float32)
    for c in range(NCORES):
        b, _g = divmod(c, 2)
        y = results[c]["y"]  # [NM, NQC, 128, CH]
        yt = y.transpose(0, 2, 1, 3).reshape(HIDDEN, S)
        out[b] += yt.T
    out += np.asarray(b_o, np.float32)[None, None, :]
    return out


def kernel(query, bias, w_qkv, b_qkv, w_o, b_o, _trace=False):
    query = np.asarray(query, np.float32)
    bias = np.asarray(bias, np.float32)
    w_qkv = np.asarray(w_qkv, np.float32)
    b_qkv = np.asarray(b_qkv, np.float32)
    w_o = np.asarray(w_o, np.float32)
    b_o = np.asarray(b_o, np.float32)

    has_bqkv = bool(np.any(b_qkv))
    has_bias = bool(np.any(bias))
    nc = _get_program(S_FULL, has_bqkv, has_bias)
    in_maps = make_in_maps(query, bias, w_qkv, b_qkv, w_o, has_bqkv, has_bias)
    res = run_bass_kernel_spmd(
        nc, in_maps, core_ids=list(range(NCORES)), trace=_trace
    )
    out = assemble_output(res.results, b_o)
    if _trace:
        return out, res
    return out


# revision 20
# speedup vs baseline: 1.0933x; 1.0035x over previous
"""Multi-head attention (B=4, S=2048, H=1024, 16 heads) on 8 TRN2 NeuronCores.

Sharding: core c handles (batch b = c//2, head-group g = c%2 of 8 heads).
Per-core device program (all activations kept feature-major, i.e. transposed):
  X^T  [1024,2048]  host-transposed bf16 query shard
  Q^T,K^T = Wq/k^T X^T            (PE, bf16, fp32 PSUM)
  V       = X W_v  (natural [s,d] via lhsT=X^T tiles)
  S^T  = per head, per k-tile: lhsT=K^T slice, rhs=Q^T slice (row-packed pairs)
  E    = exp(S^T/8)  on ScalarE, bf16 out
  O^T,sums = V'.T @ E  where V' = [V | 1] (ones column yields softmax sums)
  O^T /= sums (fast-approx reciprocal + gpsimd partition-broadcast + DVE mul)
  Y^T  = W_o^T O^T   -> DMA out per [128,512] tile

Schedule: qc (query-chunk) outer, head-pair inner.  V tiles and the next
pair's Q^T/K^T matmuls are interleaved into the attention st-loop so the
PE fills the slack under the ScalarE exp stream; the output projection for
chunk qc-1 runs under the attention of chunk qc, so only the last chunk's
projection trails.

Host: gathers per-core Y^T tiles, sums the two head-group partials per batch,
adds b_o. b_qkv / attention bias are zero in this problem; nonzero values are
still handled (extra rank-1 bias matmuls / DVE bias adds) via build flags.
"""

import numpy as np
import ml_dtypes

import concourse.bass as bass
import concourse.tile as tile
from concourse import bacc
import concourse.mybir as mybir
from concourse.bass_utils import run_bass_kernel_spmd

F32 = mybir.dt.float32
BF16 = mybir.dt.bfloat16
AF = mybir.ActivationFunctionType

HIDDEN = 1024
HEADS = 16
HD = 64
B = 4
S_FULL = 2048
NCORES = 8
HPG = HEADS // 2          # heads per group/core = 8
GF = HPG * HD             # group feature width = 512
NPAIR = HPG // 2          # head pairs per core = 4


def build_program(S=S_FULL, has_bqkv=False, has_bias=False):
    KT = HIDDEN // 128            # hidden k-tiles = 8
    CH = min(512, S)              # free-dim chunk
    NQC = S // CH                 # q chunks
    SKT = S // 128                # seq k-tiles (attention contraction)
    NM = HIDDEN // 128            # output-projection m-tiles = 8

    nc = bacc.Bacc(
        "TRN2",
        target_bir_lowering=False,
        debug=False,
        enable_asserts=False,
        num_devices=NCORES,
    )

    x_dram = nc.dram_tensor("x", [HIDDEN, S], BF16, kind="ExternalInput")  # X^T, host-transposed
    wqkv_dram = nc.dram_tensor("wqkv", [HIDDEN, 3 * GF], BF16, kind="ExternalInput")
    wo_dram = nc.dram_tensor("wo", [GF, HIDDEN], BF16, kind="ExternalInput")
    QK = 2 * GF               # q|k columns of the fused qkv weight
    if has_bqkv:
        bqkv_dram = nc.dram_tensor("bqkv", [1, 3 * GF], BF16, kind="ExternalInput")
    if has_bias:
        # host passes bias[0,0].T * 8 so exp(0.125*(S + bias8)) = exp(S/8 + bias)
        bias8_dram = nc.dram_tensor("bias8t", [S, S], F32, kind="ExternalInput")
    y_dram = nc.dram_tensor("y", [NM, NQC, 128, CH], F32, kind="ExternalOutput")

    with tile.TileContext(nc) as tc:
        with (
            tc.tile_pool(name="res", bufs=1) as res,
            tc.tile_pool(name="wrk", bufs=2) as wrk,
            tc.tile_pool(name="ep", bufs=6) as ep,
            tc.tile_pool(name="ps", bufs=2, space="PSUM") as ps,
        ):
            xt = res.tile([128, KT * S], BF16, tag="xt")
            wqkv = res.tile([128, KT * 3 * GF], BF16, tag="wqkv")
            wo = res.tile([128, (GF // 128) * HIDDEN], BF16, tag="wo")
            # V' tiles: per s-tile block of 8 heads x 65 cols (65th col = 1.0)
            vp = res.tile([128, SKT * HPG * 65], BF16, tag="vp")
            ot = res.tile([128, NPAIR * S], BF16, tag="ot")
            # Q^T/K^T for all four pairs stay live across the qc-outer loop
            qkt = res.tile([128, NPAIR * 2 * S], BF16, tag="qkt")

            nc.vector.memset(vp[:, :], 1.0)

            # DMA order follows the dependency chain of the first logits
            # tile: x chunk 0 + pair-0 q/k weight columns first (1.5 MB),
            # everything else streams in behind.  Weights ride the gpsimd
            # queue; scalar (exp) and tensor queues stay clear.
            for kt in range(KT):
                rows = slice(kt * 128, (kt + 1) * 128)
                nc.sync.dma_start(xt[:, kt * S:kt * S + CH], x_dram[rows, 0:CH])
                nc.gpsimd.dma_start(
                    wqkv[:, kt * 3 * GF:kt * 3 * GF + 128], wqkv_dram[rows, 0:128]
                )
                nc.gpsimd.dma_start(
                    wqkv[:, kt * 3 * GF + GF:kt * 3 * GF + GF + 128],
                    wqkv_dram[rows, GF:GF + 128],
                )
            for kt in range(KT):
                rows = slice(kt * 128, (kt + 1) * 128)
                nc.sync.dma_start(xt[:, kt * S + CH:(kt + 1) * S], x_dram[rows, CH:S])
                nc.gpsimd.dma_start(
                    wqkv[:, kt * 3 * GF + 2 * GF:(kt + 1) * 3 * GF],
                    wqkv_dram[rows, 2 * GF:3 * GF],
                )
            for kt in range(KT):
                rows = slice(kt * 128, (kt + 1) * 128)
                nc.gpsimd.dma_start(
                    wqkv[:, kt * 3 * GF + GF + 128:kt * 3 * GF + 2 * GF],
                    wqkv_dram[rows, GF + 128:2 * GF],
                )
                nc.gpsimd.dma_start(
                    wqkv[:, kt * 3 * GF + 128:kt * 3 * GF + GF],
                    wqkv_dram[rows, 128:GF],
                )
            for ft in range(GF // 128):
                nc.gpsimd.dma_start(
                    wo[:, ft * HIDDEN:(ft + 1) * HIDDEN],
                    wo_dram[ft * 128:(ft + 1) * 128, :],
                )
            if has_bqkv:
                bq = res.tile([1, 3 * GF], BF16, tag="bq")
                nc.sync.dma_start(bq[:, :], bqkv_dram[:, :])
                ones = res.tile([1, CH], BF16, tag="ones")
                nc.vector.memset(ones[:, :], 1.0)

            def acc_matmul(out_ps, lhsT_of, rhs_of, bias_lhsT, bias_rhs):
                """Accumulate KT matmuls (+ optional rank-1 bias term) into PSUM."""
                if bias_lhsT is not None:
                    nc.tensor.matmul(out_ps, bias_lhsT, bias_rhs, start=True, stop=False)
                for kt in range(KT):
                    nc.tensor.matmul(
                        out_ps,
                        lhsT_of(kt),
                        rhs_of(kt),
                        start=(kt == 0 and bias_lhsT is None),
                        stop=(kt == KT - 1),
                    )

            def emit_v_tile(st):
                """V' (natural [s, d]) for s-tile st, all 8 heads."""
                vps = ps.tile([128, GF], F32, tag="acc")
                acc_matmul(
                    vps[:, :],
                    lambda kt, st=st: xt[:, kt * S + st * 128: kt * S + (st + 1) * 128],
                    lambda kt: wqkv[:, kt * 3 * GF + 2 * GF: kt * 3 * GF + 3 * GF],
                    ones[0:1, 0:128] if has_bqkv else None,
                    bq[0:1, 2 * GF:3 * GF] if has_bqkv else None,
                )
                dst = vp[:, st * HPG * 65:(st + 1) * HPG * 65]
                dst = dst.rearrange("p (h c) -> p h c", c=65)[:, :, 0:64]
                src = vps.rearrange("p (h c) -> p h c", c=64)
                nc.vector.tensor_copy(dst, src)

            def emit_qk_chunk(p, which, qc):
                """One CH-wide chunk of Q^T (which=0) or K^T (which=1) for pair p."""
                colbase = p * 128 if which == 0 else GF + p * 128
                dst = qkt[:, (2 * p + which) * S + qc * CH:(2 * p + which) * S + (qc + 1) * CH]
                qkps = ps.tile([128, CH], F32, tag="acc")
                acc_matmul(
                    qkps[:, :],
                    lambda kt, cb=colbase: wqkv[:, kt * 3 * GF + cb: kt * 3 * GF + cb + 128],
                    lambda kt, qc=qc: xt[:, kt * S + qc * CH: kt * S + (qc + 1) * CH],
                    bq[0:1, colbase:colbase + 128] if has_bqkv else None,
                    ones[0:1, 0:CH] if has_bqkv else None,
                )
                nc.vector.tensor_copy(dst, qkps[:, :])

            def emit_oproj(qc, ms, tail=False):
                """Output-projection tiles m in ms for query chunk qc.  In the
                tail (no more exp work) alternate PSUM->SBUF copies between
                ScalarE and DVE so they drain twice as fast."""
                for m in ms:
                    yps = ps.tile([128, CH], F32, tag="acc")
                    for ft in range(GF // 128):
                        nc.tensor.matmul(
                            yps[:, :],
                            wo[:, ft * HIDDEN + m * 128: ft * HIDDEN + (m + 1) * 128],
                            ot[:, ft * S + qc * CH: ft * S + (qc + 1) * CH],
                            start=(ft == 0),
                            stop=(ft == GF // 128 - 1),
                        )
                    ysb = wrk.tile([128, CH], F32, tag="ysb", bufs=3)
                    if tail and m % 2 == 0:
                        nc.scalar.copy(ysb[:, :], yps[:, :])
                    else:
                        nc.vector.tensor_copy(ysb[:, :], yps[:, :])
                    nc.sync.dma_start(y_dram[m, qc], ysb[:, :])

            AV_LAG = 4  # AV trails the exp stream; must stay < ep pool bufs - 1

            def attention_unit(p, qc, extra_pe):
                """st-loop for (pair p, chunk qc); extra_pe(st) interleaves
                independent PE work (V tiles / next pair's QK / O-proj)."""
                qt = qkt[:, (2 * p + 0) * S:(2 * p + 1) * S]
                ktt = qkt[:, (2 * p + 1) * S:(2 * p + 2) * S]
                avs = []
                for j in (0, 1):
                    avp = ps.tile([65, CH], F32, tag="av", bufs=2)
                    avs.append(avp)

                def emit_av(e_tile, st):
                    # NOTE: splitting this into two row-packed K=64 halves
                    # accumulating the same PSUM (tile_position + start/stop
                    # group) hangs the device on this stack — keep K=128.
                    for j in (0, 1):
                        h = p * 2 + j
                        nc.tensor.matmul(
                            avs[j][:, :],
                            vp[:, st * HPG * 65 + h * 65: st * HPG * 65 + (h + 1) * 65],
                            e_tile[:, j * CH:(j + 1) * CH],
                            start=(st == 0),
                            stop=(st == SKT - 1),
                        )

                pending = []  # (e_tile, st) — AV trails by AV_LAG k-tiles
                for st in range(SKT):
                    if len(pending) >= AV_LAG:
                        emit_av(*pending.pop(0))
                    if st > 0:
                        # ahead of the logits emission so the in-order PE
                        # queue can chew filler while logits waits on ACT
                        extra_pe(st)
                    e = ep.tile([128, 2 * CH], BF16, tag="e")
                    sp = ps.tile([128, 2 * CH], F32, tag="sp", bufs=2)
                    for j in (0, 1):
                        hs = slice(j * 64, (j + 1) * 64)
                        spv = sp[:, j * CH:(j + 1) * CH]
                        nc.tensor.matmul(
                            spv,
                            ktt[hs, st * 128:(st + 1) * 128],
                            qt[hs, qc * CH:(qc + 1) * CH],
                            start=True,
                            stop=True,
                            tile_position=(j * 64, 0),
                        )
                        if has_bias:
                            b8 = ep.tile([128, CH], F32, tag="b8", bufs=2)
                            nc.sync.dma_start(
                                b8[:, :],
                                bias8_dram[st * 128:(st + 1) * 128, qc * CH:(qc + 1) * CH],
                            )
                            nc.vector.tensor_add(spv, spv, b8[:, :])
                    nc.scalar.activation(e[:, :], sp[:, :], AF.Exp, scale=0.125)
                    if st == 0:
                        extra_pe(st)
                    pending.append((e, st))
                for ev in pending:
                    emit_av(*ev)

                # fast PSUM->SBUF copies release both accumulator slots
                # first; the normalize chain (recip/bcast/mul) then runs
                # off the PE critical path. GpSimd runs ONLY
                # partition_broadcast (mixing gpsimd op types causes a
                # ~6us library reload per op).
                raws = []
                for j in (0, 1):
                    raw = wrk.tile([65, CH], F32, tag="raw", bufs=4)
                    nc.vector.tensor_copy(raw[:, :], avs[j][:, :])
                    raws.append(raw)
                for j in (0, 1):
                    # custom-DVE/gpsimd ops require partition-base-0 sources:
                    # shift the sums row down with a plain copy first
                    s0 = wrk.tile([1, CH], F32, tag="s0")
                    nc.vector.tensor_copy(s0[:, :], raws[j][64:65, :])
                    rec = wrk.tile([1, CH], F32, tag="rec")
                    nc.vector.reciprocal_approx_fast(rec[:, :], s0[:, :])
                    bc = wrk.tile([64, CH], F32, tag="bc", bufs=4)
                    nc.gpsimd.partition_broadcast(bc[:, :], rec[:, :])
                    nc.vector.tensor_mul(
                        ot[j * 64:(j + 1) * 64, p * S + qc * CH: p * S + (qc + 1) * CH],
                        raws[j][0:64, :],
                        bc[:, :],
                    )

            # ---- qc-outer / pair-inner schedule ----
            # prologue: only what the very first logits tile needs — K^T and
            # Q^T chunk 0 of pair 0; everything else rides in unit fillers
            emit_qk_chunk(0, 1, 0)
            emit_qk_chunk(0, 0, 0)

            def run_filler(w):
                if w[0] == "v":
                    emit_v_tile(w[1])
                elif w[0] == "qk":
                    emit_qk_chunk(w[1], w[2], w[3])
                else:
                    emit_oproj(w[1], [w[2]])

            for qc in range(NQC):
                for p in range(NPAIR):
                    filler = []
                    if qc == 0:
                        if p == 0:
                            # rest of K^T(p0): chunk c covers logits st 4c..4c+3
                            filler += [("qk", 0, 1, c) for c in range(1, NQC)]
                            # V' tiles: AV trails by AV_LAG so V[st] emitted
                            # around st/2 is always ready in time
                            filler += [("v", st) for st in range(SKT)]
                        if p + 1 < NPAIR:
                            # next pair's K^T and first Q^T chunk
                            filler += [("qk", p + 1, 1, c) for c in range(NQC)]
                            filler += [("qk", p + 1, 0, 0)]
                        # this pair's next Q^T chunk (needed at qc=1)
                        filler += [("qk", p, 0, 1)]
                    else:
                        # O-projection of previous chunk: 2 m-tiles per unit
                        filler += [("o", qc - 1, m) for m in range(2 * p, 2 * p + 2)]
                        if qc + 1 < NQC:
                            filler += [("qk", p, 0, qc + 1)]

                    # O-proj fillers wait on the previous chunk's normalize
                    # chain — start them a few st in so they don't block the
                    # in-order PE queue ahead of this unit's first logits
                    f_start = 3 if qc > 0 else 0
                    fi = iter(filler)
                    per_st = max(1, (len(filler) + SKT - f_start - 3) // (SKT - f_start - 2))

                    def extra_pe(st, fi=fi, per_st=per_st, f_start=f_start):
                        if st < f_start:
                            return
                        for _ in range(per_st):
                            w = next(fi, None)
                            if w is None:
                                return
                            run_filler(w)

                    attention_unit(p, qc, extra_pe)
                    for w in fi:
                        run_filler(w)
            emit_oproj(NQC - 1, range(NM), tail=True)

    nc.compile()
    return nc


_BUILD_CACHE = {}


def _get_program(S, has_bqkv, has_bias):
    key = (S, has_bqkv, has_bias)
    if key not in _BUILD_CACHE:
        _BUILD_CACHE[key] = build_program(S, has_bqkv, has_bias)
    return _BUILD_CACHE[key]


def make_in_maps(query, bias, w_qkv, b_qkv, w_o, has_bqkv, has_bias):
    bf = ml_dtypes.bfloat16
    in_maps = []
    for c in range(NCORES):
        b, g = divmod(c, 2)
        cols = slice(g * GF, (g + 1) * GF)
        w_g = np.concatenate(
            [w_qkv[:, cols], w_qkv[:, HIDDEN:][:, cols], w_qkv[:, 2 * HIDDEN:][:, cols]],
            axis=1,
        )
        m = {
            "x": np.ascontiguousarray(query[b].T).astype(bf),
            "wqkv": np.ascontiguousarray(w_g).astype(bf),
            "wo": np.ascontiguousarray(w_o[cols]).astype(bf),
        }
        if has_bqkv:
            b_g = np.concatenate(
                [b_qkv[cols], b_qkv[HIDDEN:][cols], b_qkv[2 * HIDDEN:][cols]]
            )
            m["bqkv"] = b_g.reshape(1, 3 * GF).astype(bf)
        if has_bias:
            m["bias8t"] = np.ascontiguousarray(bias[0, 0].T * 8.0).astype(np.float32)
        in_maps.append(m)
    return in_maps


def assemble_output(results, b_o, S=S_FULL):
    NQC = S // min(512, S)
    out = np.zeros((B, S, HIDDEN), np.float32)
    for c in range(NCORES):
        b, _g = divmod(c, 2)
        y = results[c]["y"]  # [NM, NQC, 128, CH]
        yt = y.transpose(0, 2, 1, 3).reshape(HIDDEN, S)
        out[b] += yt.T
    out += np.asarray(b_o, np.float32)[None, None, :]
    return out


def kernel(query, bias, w_qkv, b_qkv, w_o, b_o, _trace=False):
    query = np.asarray(query, np.float32)
    bias = np.asarray(bias, np.float32)
    w_qkv = np.asarray(w_qkv, np.float32)
    b_qkv = np.asarray(b_qkv, np.float32)
    w_o = np.asarray(w_o, np.float32)
    b_o = np.asarray(b_o, np.float32)

    has_bqkv = bool(np.any(b_qkv))
    has_bias = bool(np.any(bias))
    nc = _get_program(S_FULL, has_bqkv, has_bias)
    in_maps = make_in_maps(query, bias, w_qkv, b_qkv, w_o, has_bqkv, has_bias)
    res = run_bass_kernel_spmd(
        nc, in_maps, core_ids=list(range(NCORES)), trace=_trace
    )
    out = assemble_output(res.results, b_o)
    if _trace:
        return out, res
    return out
